# revision 13
# baseline (speedup 1.0000x reference)
"""BertCrf loss kernel for Trainium2 (8 NeuronCores, SPMD data-parallel).

Strategy
--------
Shapes: B=64, S=512, H=768, T=9 tags.  Loss = -sum_b(num_b - den_b).

The only heavy data is hidden_states [64,512,768] f32 (100 MB) -> the kernel
is memory-bound on streaming it once.  Each of the 8 cores takes 8 sequences.

Phase 1 (device, DMA-bound): emissions e^T [9, 4096] = fc_w @ h^T per core,
streamed from a host-pre-transposed hT [768, 4096] so the contraction dim
lands on partitions.  e^T (without fc_b; host adds it) is DMA'd back to the
host (147 KB/core) for the numerator.

Phase 2 (device): the CRF log-partition recurrence
  alpha_t[j] = logsumexp_k(alpha_{t-1,k} + trans[k,j]) + e_t[j]
is associative in the (log,+) semiring.  In linear space each step is
  P <- P @ (E * f_t[None,:]),  E = exp(trans), f_t = exp(e_t + fc_b - sigma),
so each 32-step chunk's product matrix is computed independently ->
8 seqs x 16 chunks = 128 independent 9x9 matrix chains.  These are packed
as 8 block-diagonal groups on the TensorEngine: one [72,72]x[72,144]
matmul + one [72,144] VectorE scale per step computes ALL 128 chunks.
The per-pair scale factors F are built from e^T by a small on-chip
shuffle DMA + one Exp activation.  The constant shift sigma keeps fp32 in
range (chunk log-range ~ +-45; fp32 overflows at 88).

Host (cheap, exact f64): numerator from labels + e^T; combine the 16 chunk
matrices per sequence (tiny 9x9 matvecs) with renormalization; final
logsumexp with end_transitions.  A full numpy fallback handles any
non-all-ones attention mask (the benchmark's mask is always ones).

Scan packing (hardcoded):
  pair (c, b): chunk c in [0,16), local seq b in [0,8)
  group g = c//2, c2 = c%2     -> partitions (g, j) = 8*9 = 72
  free index (c2, b, i)        -> 2*8*9 = 144 columns
  Q[(g,j), (c2,b,i)] = P_{c,b}[i, j]   (state, transposed per pair)
  step: matmul out[(g,j),n] = sum_k blockdiag(E)[(g,k),(g,j)] Q[(g,k),n],
        then Q <- out * F[(g,j),(c2,b,s)] broadcast over i.
  step s=0 is pure elementwise: Q_1 = Epat * F[...,0] where
  Epat[(g,j),(c2,b,i)] = 1.0 if chunk==0 else E[i,j]  (chunk 0's s=0 slot
  holds exp(start_j + e_0[j] + fc_b[j]) -> rows of P_0 all equal alpha_0).
"""

import numpy as np

# ---- problem constants (hardcoded per the task contract) ----
B, S, H, T = 64, 512, 768, 9
NCORES = 8
NB = B // NCORES          # 8 local sequences per core
NTOK = NB * S             # 4096 tokens per core
L = 8                     # chunk length (timesteps per chunk)
C = S // L                # 64 chunks
NPASS = 4                 # scan passes; pass p covers chunks [16p, 16p+16)
NG = 8                    # partition groups (g = c_rel//2)
P_SCAN = NG * T           # 72 scan partitions
NFREE = 2 * NB * T        # 144 scan free columns
SIGMA = 0.8               # linear-space shift (range control)
KT = H // 128             # 6 contraction tiles
TBLK = 4                  # hT column blocks (1024 tokens each)
TBC = NTOK // TBLK        # 1024 cols per block

# token order is t-major: column index = t*NB + b  (so early columns cover
# early timesteps of ALL sequences -> scan pass p only needs block p)

MOVING_DTYPE = "bf16"     # "f32" | "f32r" | "bf16"  (hidden/fc_w matmul dtype)

_cached = {}


def _np_logsumexp(x, axis):
    m = np.max(x, axis=axis, keepdims=True)
    return (m + np.log(np.sum(np.exp(x - m), axis=axis, keepdims=True))).squeeze(axis)


def _reference_host(hidden_states, attention_mask, labels, fc_w, fc_b,
                    start_transitions, end_transitions, transitions):
    """Exact numpy port of the reference (f64) - fallback for unusual inputs."""
    e = (hidden_states.astype(np.float64) @ fc_w.T.astype(np.float64)) + fc_b
    mask = attention_mask.astype(bool)
    maskf = mask.astype(np.float64)
    labels = labels.astype(np.int64)
    b_idx = np.arange(e.shape[0])

    emit = np.take_along_axis(e, labels[..., None], axis=-1)[..., 0]
    trans_sc = transitions[labels[:, :-1], labels[:, 1:]].astype(np.float64)
    num = start_transitions[labels[:, 0]].astype(np.float64) + emit[:, 0]
    num = num + ((trans_sc + emit[:, 1:]) * maskf[:, 1:]).sum(1)
    last_idx = mask.astype(np.int64).sum(1) - 1
    num = num + end_transitions[labels[b_idx, last_idx]]

    alpha = start_transitions[None, :].astype(np.float64) + e[:, 0]
    for t in range(1, e.shape[1]):
        nxt = _np_logsumexp(alpha[:, :, None] + transitions[None].astype(np.float64)
                            + e[:, t][:, None, :], axis=1)
        alpha = np.where(mask[:, t][:, None], nxt, alpha)
    den = _np_logsumexp(alpha + end_transitions[None, :].astype(np.float64), axis=1)
    return np.float32(-(num - den).sum())


def _build_nc():
    """Build the per-core Bass program (same program on all 8 cores)."""
    import concourse.bacc as bacc
    import concourse.mybir as mybir
    import concourse.tile as tile

    dt = mybir.dt
    mdt = {"f32": dt.float32, "f32r": dt.float32, "bf16": dt.bfloat16}[MOVING_DTYPE]

    # Bacc (not raw Bass): its compile() pass legalizes multi-wait sync_info
    # into what this walrus build's per-instruction wait slots accept.
    nc = bacc.Bacc("TRN2", target_bir_lowering=False, debug=False)

    hT = nc.dram_tensor("hT", [H, NTOK], mdt, kind="ExternalInput")
    fcwT = nc.dram_tensor("fcwT", [H, T], mdt, kind="ExternalInput")
    lhsE = nc.dram_tensor("lhsE", [P_SCAN, P_SCAN], dt.float32, kind="ExternalInput")
    epat = nc.dram_tensor("epat", [2, P_SCAN, NFREE], dt.float32,
                          kind="ExternalInput")    # [0]=pass0 (ones blk), [1]=rest
    biasF = nc.dram_tensor("biasF", [P_SCAN, 1], dt.float32, kind="ExternalInput")
    bias0 = nc.dram_tensor("bias0", [T, 1], dt.float32, kind="ExternalInput")
    eT_out = nc.dram_tensor("eT_out", [T, NTOK], dt.float32, kind="ExternalOutput")
    q_out = nc.dram_tensor("q_out", [P_SCAN, NPASS, NFREE], dt.float32,
                           kind="ExternalOutput")

    f32r = dt.float32r

    with tile.TileContext(nc) as tc:
        with (
            tc.tile_pool(name="const", bufs=1) as cpool,
            tc.tile_pool(name="hbuf", bufs=1) as hpool,
            tc.tile_pool(name="ebuf", bufs=1) as epool,
            tc.tile_pool(name="fbuf", bufs=2) as fpool,
            tc.tile_pool(name="scan", bufs=2) as qpool,
            tc.tile_pool(name="psum", bufs=4, space="PSUM") as pspool,
            tc.tile_pool(name="psq", bufs=4, space="PSUM") as psqpool,
        ):
            # ---- constants ----
            fcw_sb = cpool.tile([128, KT, T], mdt)
            nc.sync.dma_start(fcw_sb, fcwT.rearrange("(kt p) m -> p kt m", p=128))
            lhsE_sb = cpool.tile([P_SCAN, P_SCAN], dt.float32)
            nc.sync.dma_start(lhsE_sb, lhsE[:, :])
            epat0_sb = cpool.tile([P_SCAN, NFREE], dt.float32)
            nc.sync.dma_start(epat0_sb, epat[0])
            epatE_sb = cpool.tile([P_SCAN, NFREE], dt.float32)
            nc.sync.dma_start(epatE_sb, epat[1])
            biasF_sb = cpool.tile([P_SCAN, 1], dt.float32)
            nc.sync.dma_start(biasF_sb, biasF[:, :])
            bias0_sb = cpool.tile([T, 1], dt.float32)
            nc.sync.dma_start(bias0_sb, bias0[:, :])

            # ---- phase 1 + scan, interleaved per t-block ----
            # col = t*NB + b; block tb covers t in [128*tb, 128*(tb+1))
            eT_sb = epool.tile([T, NTOK], dt.float32)
            hT_r = hT.rearrange("(kt p) n -> kt p n", p=128)

            def do_tblock(tb):
                htiles = []
                for kt in range(KT):
                    ht = hpool.tile([128, TBC], mdt, tag=f"ht{tb}_{kt}")
                    eng = nc.sync if kt % 2 == 0 else nc.scalar
                    eng.dma_start(ht, hT_r[kt, :, tb * TBC:(tb + 1) * TBC])
                    htiles.append(ht)
                for half in range(2):
                    ps = pspool.tile([T, 512], dt.float32, tag="ps")
                    for kt in range(KT):
                        lw = fcw_sb[:, kt, :]
                        rh = htiles[kt][:, half * 512:(half + 1) * 512]
                        if MOVING_DTYPE == "f32r":
                            lw = lw.bitcast(f32r)
                            rh = rh.bitcast(f32r)
                        nc.tensor.matmul(ps, lw, rh,
                                         start=(kt == 0), stop=(kt == KT - 1))
                    nc.scalar.copy(
                        eT_sb[:, tb * TBC + half * 512: tb * TBC + half * 512 + 512],
                        ps)

            def do_scan_pass(p):
                # F[(g,j), x] = exp(eT[j, 1024p + 128g + x] + fcb_j - sigma);
                # x in [0,128) decodes as (c2, s, b) = (x//64, (x%64)//8, x%8)
                # (contiguous because token order is t-major and c_rel = 2g+c2)
                f_raw = fpool.tile([P_SCAN, 2 * NB * L], dt.float32, tag="fraw")
                for g in range(NG):
                    base = 1024 * p + 128 * g
                    nc.gpsimd.dma_start(f_raw[g * T:(g + 1) * T],
                                        eT_sb[:, base:base + 128])
                f_sb = fpool.tile([P_SCAN, 2 * NB * L], dt.float32, tag=f"f{p}")
                nc.scalar.activation(f_sb, f_raw,
                                     mybir.ActivationFunctionType.Exp, bias=biasF_sb)
                if p == 0:
                    # chunk-0 s=0 slots: alpha_0 = exp(start_j + fcb_j + e_0[b,j])
                    nc.scalar.activation(
                        f_sb[0:T, 0:NB], f_raw[0:T, 0:NB],
                        mybir.ActivationFunctionType.Exp, bias=bias0_sb)
                # scan chain for this pass; q free layout (c2, b, i)
                f_v = f_sb.rearrange("p (c2 s b) -> p c2 s b", c2=2, s=L)
                q = qpool.tile([P_SCAN, 2, NB, T], dt.float32, tag=f"q{p}")

                def fslice(s):
                    # [72, (c2,b,i)] view of F at step s, broadcast over i
                    return f_v[:, :, s, :].unsqueeze(-1).broadcast_to(
                        [P_SCAN, 2, NB, T])

                ep = epat0_sb if p == 0 else epatE_sb
                nc.vector.tensor_mul(
                    q, ep.rearrange("p (c2 b i) -> p c2 b i", c2=2, b=NB), fslice(0))
                for s in range(1, L):
                    psq = psqpool.tile([P_SCAN, NFREE], dt.float32, tag="psq")
                    nc.tensor.matmul(psq, lhsE_sb,
                                     q.rearrange("p c2 b i -> p (c2 b i)"),
                                     start=True, stop=True)
                    qn = qpool.tile([P_SCAN, 2, NB, T], dt.float32, tag=f"q{p}")
                    nc.vector.tensor_mul(
                        qn, psq.rearrange("p (c2 b i) -> p c2 b i", c2=2, b=NB),
                        fslice(s))
                    q = qn
                nc.gpsimd.dma_start(q_out[:, p, :],
                                    q.rearrange("p c2 b i -> p (c2 b i)"))

            for tb in range(TBLK):
                do_tblock(tb)
                do_scan_pass(tb)
            nc.gpsimd.dma_start(eT_out[:, :], eT_sb)

    nc.compile()
    return nc


def _get_nc():
    if "nc" not in _cached:
        _cached["nc"] = _build_nc()
    return _cached["nc"]


def _host_prep(hidden_states, fc_w, fc_b, start_transitions, transitions):
    """Build the 8 per-core input maps."""
    if MOVING_DTYPE == "bf16":
        import ml_dtypes
        np_mdt = ml_dtypes.bfloat16
    else:
        np_mdt = np.float32

    E = np.exp(transitions.astype(np.float64)).astype(np.float32)     # [T,T]
    # epat[e][(g,j),(c2,b,i)] = E[i,j], with e=0 having chunk-0 (g0,c2=0) = 1
    epatE = np.tile(E.T[None, :, None, None, :], (NG, 1, 2, NB, 1))   # [g,j,c2,b,i]
    epat0 = epatE.copy()
    epat0[0, :, 0, :, :] = 1.0
    epat = np.ascontiguousarray(np.stack([
        epat0.reshape(P_SCAN, NFREE), epatE.reshape(P_SCAN, NFREE)]),
        dtype=np.float32)                                             # [2,72,144]
    # lhsE = blockdiag(E) x8: lhsT[(g,k),(g,j)] = E[k,j]
    lhsE = np.zeros((P_SCAN, P_SCAN), dtype=np.float32)
    for g in range(NG):
        lhsE[g * T:(g + 1) * T, g * T:(g + 1) * T] = E
    fcwT = np.ascontiguousarray(fc_w.T.astype(np_mdt))                # [H,T]
    biasF = np.ascontiguousarray(
        np.tile(fc_b - SIGMA, NG).reshape(P_SCAN, 1), dtype=np.float32)
    bias0 = np.ascontiguousarray(
        (start_transitions + fc_b).reshape(T, 1), dtype=np.float32)

    in_maps = []
    for cid in range(NCORES):
        hc = hidden_states[cid * NB:(cid + 1) * NB]                   # [NB,S,H]
        # t-major token order: col = t*NB + b
        hc = hc.transpose(1, 0, 2).reshape(NTOK, H)
        hTc = np.ascontiguousarray(hc.T.astype(np_mdt))               # [H,4096]
        in_maps.append({
            "hT": hTc, "fcwT": fcwT, "lhsE": lhsE, "epat": epat,
            "biasF": biasF, "bias0": bias0,
        })
    return in_maps


def _host_finish(results, labels, fc_b, start_transitions,
                 end_transitions, transitions):
    """Numerator + chunk-matrix combine, all in f64."""
    labels = labels.astype(np.int64)
    start = start_transitions.astype(np.float64)
    end = end_transitions.astype(np.float64)
    trans = transitions.astype(np.float64)

    # reassemble e [B, S, T] from per-core e^T [9, 4096] (+ fc_b)
    # token order is t-major: col = t*NB + b
    e = np.empty((B, S, T), dtype=np.float64)
    for cid in range(NCORES):
        eT = results[cid]["eT_out"].astype(np.float64)    # [9, 4096]
        e[cid * NB:(cid + 1) * NB] = eT.T.reshape(S, NB, T).transpose(1, 0, 2)
    e += fc_b.astype(np.float64)

    # numerator (mask all-ones fast path)
    emit = np.take_along_axis(e, labels[..., None], axis=-1)[..., 0]
    num = start[labels[:, 0]] + emit[:, 0]
    num = num + (trans[labels[:, :-1], labels[:, 1:]] + emit[:, 1:]).sum(1)
    num = num + end[labels[:, -1]]

    # denominator: combine chunk matrices
    # chunk c_abs = 16p + 2g + c2;  Q[(g,j), p, (c2,b,i)] = P_c[i, j]
    den = np.empty(B)
    for cid in range(NCORES):
        Q = results[cid]["q_out"].astype(np.float64)      # [72, NPASS, 144]
        Q = Q.reshape(NG, T, NPASS, 2, NB, T)             # [g, j, p, c2, b, i]
        for b in range(NB):
            alpha = Q[0, :, 0, 0, b, 0].copy()  # P_0[0,:] (rows of P_0 all equal)
            corr = 0.0
            for c in range(1, C):
                p, c_rel = c // 16, c % 16
                g, c2 = c_rel // 2, c_rel % 2
                Pc = Q[g, :, p, c2, b, :].T               # P_c[i, j] rows i
                alpha = alpha @ Pc
                m = alpha.max()
                alpha /= m
                corr += np.log(m)
            den[cid * NB + b] = np.log((alpha * np.exp(end)).sum()) + corr \
                + (S - 1) * SIGMA
    return np.float32(-(num - den).sum())


def kernel(**inputs):
    hidden_states = np.asarray(inputs["hidden_states"], dtype=np.float32)
    attention_mask = np.asarray(inputs["attention_mask"])
    labels = np.asarray(inputs["labels"])
    fc_w = np.asarray(inputs["fc_w"], dtype=np.float32)
    fc_b = np.asarray(inputs["fc_b"], dtype=np.float32)
    start_transitions = np.asarray(inputs["start_transitions"], dtype=np.float32)
    end_transitions = np.asarray(inputs["end_transitions"], dtype=np.float32)
    transitions = np.asarray(inputs["transitions"], dtype=np.float32)

    if (hidden_states.shape != (B, S, H)) or not np.all(attention_mask != 0):
        return _reference_host(hidden_states, attention_mask, labels, fc_w,
                               fc_b, start_transitions, end_transitions,
                               transitions)

    from concourse.bass_utils import run_bass_kernel_spmd
    nc = _get_nc()
    in_maps = _host_prep(hidden_states, fc_w, fc_b, start_transitions,
                         transitions)
    res = run_bass_kernel_spmd(nc, in_maps, core_ids=list(range(NCORES)))
    _cached["last_res"] = res
    return _host_finish(res.results, labels, fc_b, start_transitions,
                        end_transitions, transitions)


# revision 29
# speedup vs baseline: 1.0614x; 1.0614x over previous
"""BertCrf loss kernel for Trainium2 (8 NeuronCores, SPMD data-parallel).

Strategy
--------
Shapes: B=64, S=512, H=768, T=9 tags.  Loss = -sum_b(num_b - den_b).

The only heavy data is hidden_states [64,512,768] f32 (100 MB) -> the kernel
is memory-bound on streaming it once.  Each of the 8 cores takes 8 sequences.

Phase 1 (device, DMA-bound): emissions e^T [9, 4096] = fc_w @ h^T per core,
streamed from a host-pre-transposed hT [768, 4096] so the contraction dim
lands on partitions.  e^T (without fc_b; host adds it) is DMA'd back to the
host (147 KB/core) for the numerator.

Phase 2 (device): the CRF log-partition recurrence
  alpha_t[j] = logsumexp_k(alpha_{t-1,k} + trans[k,j]) + e_t[j]
is associative in the (log,+) semiring.  In linear space each step is
  P <- P @ (E * f_t[None,:]),  E = exp(trans), f_t = exp(e_t + fc_b - sigma),
so each 32-step chunk's product matrix is computed independently ->
8 seqs x 16 chunks = 128 independent 9x9 matrix chains.  These are packed
as 8 block-diagonal groups on the TensorEngine: one [72,72]x[72,144]
matmul + one [72,144] VectorE scale per step computes ALL 128 chunks.
The per-pair scale factors F are built from e^T by a small on-chip
shuffle DMA + one Exp activation.  The constant shift sigma keeps fp32 in
range (chunk log-range ~ +-45; fp32 overflows at 88).

Host (cheap, exact f64): numerator from labels + e^T; combine the 16 chunk
matrices per sequence (tiny 9x9 matvecs) with renormalization; final
logsumexp with end_transitions.  A full numpy fallback handles any
non-all-ones attention mask (the benchmark's mask is always ones).

Scan packing (hardcoded):
  pair (c, b): chunk c in [0,16), local seq b in [0,8)
  group g = c//2, c2 = c%2     -> partitions (g, j) = 8*9 = 72
  free index (c2, b, i)        -> 2*8*9 = 144 columns
  Q[(g,j), (c2,b,i)] = P_{c,b}[i, j]   (state, transposed per pair)
  step: matmul out[(g,j),n] = sum_k blockdiag(E)[(g,k),(g,j)] Q[(g,k),n],
        then Q <- out * F[(g,j),(c2,b,s)] broadcast over i.
  step s=0 is pure elementwise: Q_1 = Epat * F[...,0] where
  Epat[(g,j),(c2,b,i)] = 1.0 if chunk==0 else E[i,j]  (chunk 0's s=0 slot
  holds exp(start_j + e_0[j] + fc_b[j]) -> rows of P_0 all equal alpha_0).
"""

import numpy as np

# ---- problem constants (hardcoded per the task contract) ----
B, S, H, T = 64, 512, 768, 9
NCORES = 8
NB = B // NCORES          # 8 local sequences per core
NTOK = NB * S             # 4096 tokens per core
L = 8                     # chunk length (timesteps per chunk)
C = S // L                # 64 chunks
NPASS = 4                 # scan passes; pass p covers chunks [16p, 16p+16)
NG = 8                    # partition groups (g = c_rel//2)
P_SCAN = NG * T           # 72 scan partitions
NFREE = 2 * NB * T        # 144 scan free columns
SIGMA = 0.8               # linear-space shift (range control)
KT = H // 128             # 6 contraction tiles
TBLK = 4                  # hT column blocks (1024 tokens each)
TBC = NTOK // TBLK        # 1024 cols per block

# token order is t-major: column index = t*NB + b  (so early columns cover
# early timesteps of ALL sequences -> scan pass p only needs block p)

MOVING_DTYPE = "bf16"     # "f32" | "f32r" | "bf16"  (hidden/fc_w matmul dtype)

_cached = {}


def _np_logsumexp(x, axis):
    m = np.max(x, axis=axis, keepdims=True)
    return (m + np.log(np.sum(np.exp(x - m), axis=axis, keepdims=True))).squeeze(axis)


def _reference_host(hidden_states, attention_mask, labels, fc_w, fc_b,
                    start_transitions, end_transitions, transitions):
    """Exact numpy port of the reference (f64) - fallback for unusual inputs."""
    e = (hidden_states.astype(np.float64) @ fc_w.T.astype(np.float64)) + fc_b
    mask = attention_mask.astype(bool)
    maskf = mask.astype(np.float64)
    labels = labels.astype(np.int64)
    b_idx = np.arange(e.shape[0])

    emit = np.take_along_axis(e, labels[..., None], axis=-1)[..., 0]
    trans_sc = transitions[labels[:, :-1], labels[:, 1:]].astype(np.float64)
    num = start_transitions[labels[:, 0]].astype(np.float64) + emit[:, 0]
    num = num + ((trans_sc + emit[:, 1:]) * maskf[:, 1:]).sum(1)
    last_idx = mask.astype(np.int64).sum(1) - 1
    num = num + end_transitions[labels[b_idx, last_idx]]

    alpha = start_transitions[None, :].astype(np.float64) + e[:, 0]
    for t in range(1, e.shape[1]):
        nxt = _np_logsumexp(alpha[:, :, None] + transitions[None].astype(np.float64)
                            + e[:, t][:, None, :], axis=1)
        alpha = np.where(mask[:, t][:, None], nxt, alpha)
    den = _np_logsumexp(alpha + end_transitions[None, :].astype(np.float64), axis=1)
    return np.float32(-(num - den).sum())


def _build_nc():
    """Build the per-core Bass program (same program on all 8 cores)."""
    import concourse.bacc as bacc
    import concourse.mybir as mybir
    import concourse.tile as tile

    dt = mybir.dt
    mdt = {"f32": dt.float32, "f32r": dt.float32, "bf16": dt.bfloat16}[MOVING_DTYPE]

    # Bacc (not raw Bass): its compile() pass legalizes multi-wait sync_info
    # into what this walrus build's per-instruction wait slots accept.
    nc = bacc.Bacc("TRN2", target_bir_lowering=False, debug=False)

    hT = nc.dram_tensor("hT", [H, NTOK], mdt, kind="ExternalInput")
    fcwT = nc.dram_tensor("fcwT", [H, T], mdt, kind="ExternalInput")
    lhsE = nc.dram_tensor("lhsE", [P_SCAN, P_SCAN], dt.bfloat16, kind="ExternalInput")
    epat = nc.dram_tensor("epat", [2, P_SCAN, NFREE], dt.float32,
                          kind="ExternalInput")    # [0]=pass0 (ones blk), [1]=rest
    biasF = nc.dram_tensor("biasF", [P_SCAN, 1], dt.float32, kind="ExternalInput")
    bias0 = nc.dram_tensor("bias0", [T, 1], dt.float32, kind="ExternalInput")
    eT_out = nc.dram_tensor("eT_out", [T, NTOK], dt.float32, kind="ExternalOutput")
    q_out = nc.dram_tensor("q_out", [P_SCAN, NPASS, NFREE], dt.float32,
                           kind="ExternalOutput")

    f32r = dt.float32r

    with tile.TileContext(nc) as tc:
        with (
            tc.tile_pool(name="const", bufs=1) as cpool,
            tc.tile_pool(name="hbuf", bufs=1) as hpool,
            tc.tile_pool(name="ebuf", bufs=1) as epool,
            tc.tile_pool(name="fbuf", bufs=2) as fpool,
            tc.tile_pool(name="scan", bufs=2) as qpool,
        ):
            # ---- constants ----
            fcw_sb = cpool.tile([128, KT, T], mdt)
            nc.sync.dma_start(fcw_sb, fcwT.rearrange("(kt p) m -> p kt m", p=128))
            lhsE_sb = cpool.tile([P_SCAN, P_SCAN], dt.bfloat16)
            nc.sync.dma_start(lhsE_sb, lhsE[:, :])
            epat0_sb = cpool.tile([P_SCAN, NFREE], dt.float32)
            nc.sync.dma_start(epat0_sb, epat[0])
            epatE_sb = cpool.tile([P_SCAN, NFREE], dt.float32)
            nc.sync.dma_start(epatE_sb, epat[1])
            biasF_sb = cpool.tile([P_SCAN, 1], dt.float32)
            nc.sync.dma_start(biasF_sb, biasF[:, :])
            bias0_sb = cpool.tile([T, 1], dt.float32)
            nc.sync.dma_start(bias0_sb, bias0[:, :])

            # ---- phase 1: 6 full-row 1MB DMAs; kt-outer matmuls into
            # partition-packed PSUM (two banks, 4 x 32-aligned slots each) ----
            eT_sb = epool.tile([T, NTOK], dt.float32)
            hT_r = hT.rearrange("(kt p) n -> kt p n", p=128)
            htiles = []
            for kt in range(KT):
                ht = hpool.tile([128, NTOK], mdt, tag=f"ht{kt}")
                eng = nc.sync if kt % 2 == 0 else nc.scalar
                eng.dma_start(ht, hT_r[kt])
                htiles.append(ht)
            # 8 accumulators, one PSUM bank each; kt-outer so the PE chases
            # the 6 row DMAs instead of stalling on the last one
            with tc.tile_pool(name="psum1", bufs=1, space="PSUM") as pspool:
                psbank = [pspool.tile([T, 512], dt.float32, tag=f"psb{i}",
                                      name=f"psb{i}")
                          for i in range(8)]
                for kt in range(KT):
                    for n in range(8):
                        lw = fcw_sb[:, kt, :]
                        rh = htiles[kt][:, n * 512:(n + 1) * 512]
                        if MOVING_DTYPE == "f32r":
                            lw = lw.bitcast(f32r)
                            rh = rh.bitcast(f32r)
                        nc.tensor.matmul(psbank[n], lw, rh,
                                         start=(kt == 0), stop=(kt == KT - 1))
                ecopies = []
                for n in range(8):
                    ecopies.append(
                        nc.any.tensor_copy(eT_sb[:, n * 512:(n + 1) * 512],
                                           psbank[n]))

            # ---- F build (all passes at once): 8 strided per-group DMAs ----
            # F_all[(g,j), (p, x)] = eT[j, 1024p + 128g + x];
            # x in [0,128) decodes as (c2, s, b) = (x//64, (x%64)//8, x%8)
            # (contiguous because token order is t-major and c_rel = 2g+c2)
            eT_pgx = eT_sb.rearrange("j (p g x) -> j p g x", p=NPASS, g=NG)
            f_raw = fpool.tile([P_SCAN, NPASS, 2 * NB * L], dt.float32)
            for g in range(NG):
                nc.gpsimd.dma_start(f_raw[g * T:(g + 1) * T],
                                    eT_pgx[:, :, g, :])
            f_all = fpool.tile([P_SCAN, NPASS, 2 * NB * L], dt.float32)
            nc.scalar.activation(f_all, f_raw,
                                 mybir.ActivationFunctionType.Exp, bias=biasF_sb)
            # chunk-0 s=0 slots: alpha_0 = exp(start_j + fcb_j + e_0[b,j])
            nc.scalar.activation(
                f_all[0:T, 0, 0:NB], f_raw[0:T, 0, 0:NB],
                mybir.ActivationFunctionType.Exp, bias=bias0_sb)

            def do_scan_pass(p, psqpool):
                # scan chain for this pass; q free layout (c2, b, i); bf16 state
                f_v = f_all[:, p].rearrange("p (c2 s b) -> p c2 s b", c2=2, s=L)
                q = qpool.tile([P_SCAN, 2, NB, T], dt.bfloat16, tag=f"q{p}")

                def fslice(s):
                    # [72, (c2,b,i)] view of F at step s, broadcast over i
                    return f_v[:, :, s, :].unsqueeze(-1).broadcast_to(
                        [P_SCAN, 2, NB, T])

                ep = epat0_sb if p == 0 else epatE_sb
                nc.vector.tensor_mul(
                    q, ep.rearrange("p (c2 b i) -> p c2 b i", c2=2, b=NB), fslice(0))
                for s in range(1, L):
                    psq = psqpool.tile([P_SCAN, NFREE], dt.float32, tag="psq")
                    nc.tensor.matmul(psq, lhsE_sb,
                                     q.rearrange("p c2 b i -> p (c2 b i)"),
                                     start=True, stop=True)
                    qn = qpool.tile([P_SCAN, 2, NB, T], dt.bfloat16, tag=f"q{p}")
                    nc.vector.tensor_mul(
                        qn, psq.rearrange("p (c2 b i) -> p c2 b i", c2=2, b=NB),
                        fslice(s))
                    q = qn
                # final chunk matrices for this pass, back to f32 for the host
                qf = fpool.tile([P_SCAN, NFREE], dt.float32, tag=f"qf{p}")
                nc.any.tensor_copy(qf, q.rearrange("p c2 b i -> p (c2 b i)"))
                nc.gpsimd.dma_start(q_out[:, p, :], qf)

            with tc.tile_pool(name="psq", bufs=6, space="PSUM") as psqpool:
                for p in range(NPASS):
                    do_scan_pass(p, psqpool)
            nc.gpsimd.dma_start(eT_out[:, :], eT_sb)

    nc.compile()
    return nc


def _get_nc():
    if "nc" not in _cached:
        _cached["nc"] = _build_nc()
    return _cached["nc"]


def _host_prep(hidden_states, fc_w, fc_b, start_transitions, transitions):
    """Build the 8 per-core input maps."""
    import ml_dtypes
    np_mdt = ml_dtypes.bfloat16 if MOVING_DTYPE == "bf16" else np.float32

    E = np.exp(transitions.astype(np.float64)).astype(np.float32)     # [T,T]
    # epat[e][(g,j),(c2,b,i)] = E[i,j], with e=0 having chunk-0 (g0,c2=0) = 1
    epatE = np.tile(E.T[None, :, None, None, :], (NG, 1, 2, NB, 1))   # [g,j,c2,b,i]
    epat0 = epatE.copy()
    epat0[0, :, 0, :, :] = 1.0
    epat = np.ascontiguousarray(np.stack([
        epat0.reshape(P_SCAN, NFREE), epatE.reshape(P_SCAN, NFREE)]),
        dtype=np.float32)                                             # [2,72,144]
    # lhsE = blockdiag(E) x8: lhsT[(g,k),(g,j)] = E[k,j]  (bf16 scan matmul)
    lhsE = np.zeros((P_SCAN, P_SCAN), dtype=ml_dtypes.bfloat16)
    for g in range(NG):
        lhsE[g * T:(g + 1) * T, g * T:(g + 1) * T] = E.astype(ml_dtypes.bfloat16)
    fcwT = np.ascontiguousarray(fc_w.T.astype(np_mdt))                # [H,T]
    biasF = np.ascontiguousarray(
        np.tile(fc_b - SIGMA, NG).reshape(P_SCAN, 1), dtype=np.float32)
    bias0 = np.ascontiguousarray(
        (start_transitions + fc_b).reshape(T, 1), dtype=np.float32)

    in_maps = []
    for cid in range(NCORES):
        hc = hidden_states[cid * NB:(cid + 1) * NB]                   # [NB,S,H]
        # t-major token order: col = t*NB + b
        hc = hc.transpose(1, 0, 2).reshape(NTOK, H)
        hTc = np.ascontiguousarray(hc.T.astype(np_mdt))               # [H,4096]
        in_maps.append({
            "hT": hTc, "fcwT": fcwT, "lhsE": lhsE, "epat": epat,
            "biasF": biasF, "bias0": bias0,
        })
    return in_maps


def _host_finish(results, labels, fc_b, start_transitions,
                 end_transitions, transitions):
    """Numerator + chunk-matrix combine, all in f64."""
    labels = labels.astype(np.int64)
    start = start_transitions.astype(np.float64)
    end = end_transitions.astype(np.float64)
    trans = transitions.astype(np.float64)

    # reassemble e [B, S, T] from per-core e^T [9, 4096] (+ fc_b)
    # token order is t-major: col = t*NB + b
    e = np.empty((B, S, T), dtype=np.float64)
    for cid in range(NCORES):
        eT = results[cid]["eT_out"].astype(np.float64)    # [9, 4096]
        e[cid * NB:(cid + 1) * NB] = eT.T.reshape(S, NB, T).transpose(1, 0, 2)
    e += fc_b.astype(np.float64)

    # numerator (mask all-ones fast path)
    emit = np.take_along_axis(e, labels[..., None], axis=-1)[..., 0]
    num = start[labels[:, 0]] + emit[:, 0]
    num = num + (trans[labels[:, :-1], labels[:, 1:]] + emit[:, 1:]).sum(1)
    num = num + end[labels[:, -1]]

    # denominator: combine chunk matrices
    # chunk c_abs = 16p + 2g + c2;  Q[(g,j), p, (c2,b,i)] = P_c[i, j]
    den = np.empty(B)
    for cid in range(NCORES):
        Q = results[cid]["q_out"].astype(np.float64)      # [72, NPASS, 144]
        Q = Q.reshape(NG, T, NPASS, 2, NB, T)             # [g, j, p, c2, b, i]
        for b in range(NB):
            alpha = Q[0, :, 0, 0, b, 0].copy()  # P_0[0,:] (rows of P_0 all equal)
            corr = 0.0
            for c in range(1, C):
                p, c_rel = c // 16, c % 16
                g, c2 = c_rel // 2, c_rel % 2
                Pc = Q[g, :, p, c2, b, :].T               # P_c[i, j] rows i
                alpha = alpha @ Pc
                m = alpha.max()
                alpha /= m
                corr += np.log(m)
            den[cid * NB + b] = np.log((alpha * np.exp(end)).sum()) + corr \
                + (S - 1) * SIGMA
    return np.float32(-(num - den).sum())


def kernel(**inputs):
    hidden_states = np.asarray(inputs["hidden_states"], dtype=np.float32)
    attention_mask = np.asarray(inputs["attention_mask"])
    labels = np.asarray(inputs["labels"])
    fc_w = np.asarray(inputs["fc_w"], dtype=np.float32)
    fc_b = np.asarray(inputs["fc_b"], dtype=np.float32)
    start_transitions = np.asarray(inputs["start_transitions"], dtype=np.float32)
    end_transitions = np.asarray(inputs["end_transitions"], dtype=np.float32)
    transitions = np.asarray(inputs["transitions"], dtype=np.float32)

    if (hidden_states.shape != (B, S, H)) or not np.all(attention_mask != 0):
        return _reference_host(hidden_states, attention_mask, labels, fc_w,
                               fc_b, start_transitions, end_transitions,
                               transitions)

    from concourse.bass_utils import run_bass_kernel_spmd
    nc = _get_nc()
    in_maps = _host_prep(hidden_states, fc_w, fc_b, start_transitions,
                         transitions)
    res = run_bass_kernel_spmd(nc, in_maps, core_ids=list(range(NCORES)))
    _cached["last_res"] = res
    return _host_finish(res.results, labels, fc_b, start_transitions,
                        end_transitions, transitions)


# revision 36
# speedup vs baseline: 1.1867x; 1.1181x over previous
"""BertCrf loss kernel for Trainium2 (8 NeuronCores, SPMD data-parallel).

Strategy
--------
Shapes: B=64, S=512, H=768, T=9 tags.  Loss = -sum_b(num_b - den_b).

The only heavy data is hidden_states [64,512,768] f32 (100 MB) -> the kernel
is memory-bound on streaming it once.  Each of the 8 cores takes 8 sequences.

Phase 1 (device, DMA-bound): emissions e^T [9, 4096] = fc_w @ h^T per core,
streamed from a host-pre-transposed hT [768, 4096] so the contraction dim
lands on partitions.  e^T (without fc_b; host adds it) is DMA'd back to the
host (147 KB/core) for the numerator.

Phase 2 (device): the CRF log-partition recurrence
  alpha_t[j] = logsumexp_k(alpha_{t-1,k} + trans[k,j]) + e_t[j]
is associative in the (log,+) semiring.  In linear space each step is
  P <- P @ (E * f_t[None,:]),  E = exp(trans), f_t = exp(e_t + fc_b - sigma),
so each 32-step chunk's product matrix is computed independently ->
8 seqs x 16 chunks = 128 independent 9x9 matrix chains.  These are packed
as 8 block-diagonal groups on the TensorEngine: one [72,72]x[72,144]
matmul + one [72,144] VectorE scale per step computes ALL 128 chunks.
The per-pair scale factors F are built from e^T by a small on-chip
shuffle DMA + one Exp activation.  The constant shift sigma keeps fp32 in
range (chunk log-range ~ +-45; fp32 overflows at 88).

Host (cheap, exact f64): numerator from labels + e^T; combine the 16 chunk
matrices per sequence (tiny 9x9 matvecs) with renormalization; final
logsumexp with end_transitions.  A full numpy fallback handles any
non-all-ones attention mask (the benchmark's mask is always ones).

Scan packing (hardcoded):
  pair (c, b): chunk c in [0,16), local seq b in [0,8)
  group g = c//2, c2 = c%2     -> partitions (g, j) = 8*9 = 72
  free index (c2, b, i)        -> 2*8*9 = 144 columns
  Q[(g,j), (c2,b,i)] = P_{c,b}[i, j]   (state, transposed per pair)
  step: matmul out[(g,j),n] = sum_k blockdiag(E)[(g,k),(g,j)] Q[(g,k),n],
        then Q <- out * F[(g,j),(c2,b,s)] broadcast over i.
  step s=0 is pure elementwise: Q_1 = Epat * F[...,0] where
  Epat[(g,j),(c2,b,i)] = 1.0 if chunk==0 else E[i,j]  (chunk 0's s=0 slot
  holds exp(start_j + e_0[j] + fc_b[j]) -> rows of P_0 all equal alpha_0).
"""

import numpy as np

# ---- problem constants (hardcoded per the task contract) ----
B, S, H, T = 64, 512, 768, 9
NCORES = 8
NB = B // NCORES          # 8 local sequences per core
NTOK = NB * S             # 4096 tokens per core
L = 8                     # chunk length (timesteps per chunk)
C = S // L                # 64 chunks
NG = 8                    # partition groups; chunk c = 8g + c3
C3 = 8                    # chunks per group
P_SCAN = NG * T           # 72 scan partitions
NFREE = C3 * NB * T       # 576 scan free columns (c3, b, i)
SIGMA = 0.8               # linear-space shift (range control)
KT = H // 128             # 6 contraction tiles

# token order is t-major: column index = t*NB + b.  Then PSUM bank g of the
# emissions matmul (columns [512g, 512g+512)) is exactly scan group g's
# (c3, s, b) panel: col = 512g + 64*c3 + 8*s + b.

MOVING_DTYPE = "bf16"     # "f32" | "f32r" | "bf16"  (hidden/fc_w matmul dtype)

_cached = {}


def _np_logsumexp(x, axis):
    m = np.max(x, axis=axis, keepdims=True)
    return (m + np.log(np.sum(np.exp(x - m), axis=axis, keepdims=True))).squeeze(axis)


def _reference_host(hidden_states, attention_mask, labels, fc_w, fc_b,
                    start_transitions, end_transitions, transitions):
    """Exact numpy port of the reference (f64) - fallback for unusual inputs."""
    e = (hidden_states.astype(np.float64) @ fc_w.T.astype(np.float64)) + fc_b
    mask = attention_mask.astype(bool)
    maskf = mask.astype(np.float64)
    labels = labels.astype(np.int64)
    b_idx = np.arange(e.shape[0])

    emit = np.take_along_axis(e, labels[..., None], axis=-1)[..., 0]
    trans_sc = transitions[labels[:, :-1], labels[:, 1:]].astype(np.float64)
    num = start_transitions[labels[:, 0]].astype(np.float64) + emit[:, 0]
    num = num + ((trans_sc + emit[:, 1:]) * maskf[:, 1:]).sum(1)
    last_idx = mask.astype(np.int64).sum(1) - 1
    num = num + end_transitions[labels[b_idx, last_idx]]

    alpha = start_transitions[None, :].astype(np.float64) + e[:, 0]
    for t in range(1, e.shape[1]):
        nxt = _np_logsumexp(alpha[:, :, None] + transitions[None].astype(np.float64)
                            + e[:, t][:, None, :], axis=1)
        alpha = np.where(mask[:, t][:, None], nxt, alpha)
    den = _np_logsumexp(alpha + end_transitions[None, :].astype(np.float64), axis=1)
    return np.float32(-(num - den).sum())


def _build_nc():
    """Build the per-core Bass program (same program on all 8 cores)."""
    import concourse.bacc as bacc
    import concourse.mybir as mybir
    import concourse.tile as tile

    dt = mybir.dt
    mdt = {"f32": dt.float32, "f32r": dt.float32, "bf16": dt.bfloat16}[MOVING_DTYPE]

    # Bacc (not raw Bass): its compile() pass legalizes multi-wait sync_info
    # into what this walrus build's per-instruction wait slots accept.
    nc = bacc.Bacc("TRN2", target_bir_lowering=False, debug=False)

    hT = nc.dram_tensor("hT", [H, NTOK], mdt, kind="ExternalInput")
    fcwT = nc.dram_tensor("fcwT", [H, T], mdt, kind="ExternalInput")
    lhsE = nc.dram_tensor("lhsE", [P_SCAN, P_SCAN], dt.bfloat16, kind="ExternalInput")
    epat = nc.dram_tensor("epat", [P_SCAN, NFREE], dt.float32, kind="ExternalInput")
    biasF = nc.dram_tensor("biasF", [P_SCAN, 1], dt.float32, kind="ExternalInput")
    bias0 = nc.dram_tensor("bias0", [T, 1], dt.float32, kind="ExternalInput")
    eT_out = nc.dram_tensor("eT_out", [T, NTOK], dt.float32, kind="ExternalOutput")
    q_out = nc.dram_tensor("q_out", [P_SCAN, NFREE], dt.float32,
                           kind="ExternalOutput")

    f32r = dt.float32r
    HALF = NFREE // 2          # 288 free columns per scan half-chain

    with tile.TileContext(nc) as tc:
        with (
            tc.tile_pool(name="const", bufs=1) as cpool,
            tc.tile_pool(name="hbuf", bufs=1) as hpool,
            tc.tile_pool(name="fbuf", bufs=1) as fpool,
            tc.tile_pool(name="scan", bufs=2) as qpool,
        ):
            # ---- constants (SWDGE so the HWDGE rings start hT immediately) ----
            fcw_sb = cpool.tile([128, KT, T], mdt)
            nc.gpsimd.dma_start(fcw_sb, fcwT.rearrange("(kt p) m -> p kt m", p=128))
            lhsE_sb = cpool.tile([P_SCAN, P_SCAN], dt.bfloat16)
            nc.gpsimd.dma_start(lhsE_sb, lhsE[:, :])
            epat_sb = cpool.tile([P_SCAN, NFREE], dt.float32)
            nc.gpsimd.dma_start(epat_sb, epat[:, :])
            biasF_sb = cpool.tile([P_SCAN, 1], dt.float32)
            nc.gpsimd.dma_start(biasF_sb, biasF[:, :])
            bias0_sb = cpool.tile([T, 1], dt.float32)
            nc.gpsimd.dma_start(bias0_sb, bias0[:, :])

            # ---- phase 1: 6 full-row 1MB DMAs split across both HWDGE rings;
            # kt-outer matmuls so the PE chases the row DMAs ----
            hT_r = hT.rearrange("(kt p) n -> kt p n", p=128)
            htiles = []
            for kt in range(KT):
                ht = hpool.tile([128, NTOK], mdt, tag=f"ht{kt}", name=f"ht{kt}")
                eng = nc.sync if kt % 2 == 0 else nc.scalar
                eng.dma_start(ht, hT_r[kt])
                htiles.append(ht)

            f_raw = fpool.tile([P_SCAN, C3 * L * NB], dt.float32)
            eT_sb = fpool.tile([T, NTOK], dt.float32)
            with tc.tile_pool(name="psum1", bufs=1, space="PSUM") as pspool:
                psbank = [pspool.tile([T, 512], dt.float32, tag=f"psb{i}",
                                      name=f"psb{i}")
                          for i in range(8)]
                for kt in range(KT):
                    for n in range(8):
                        lw = fcw_sb[:, kt, :]
                        rh = htiles[kt][:, n * 512:(n + 1) * 512]
                        if MOVING_DTYPE == "f32r":
                            lw = lw.bitcast(f32r)
                            rh = rh.bitcast(f32r)
                        nc.tensor.matmul(psbank[n], lw, rh,
                                         start=(kt == 0), stop=(kt == KT - 1))
                # PSUM -> SBUF (DMA cannot read PSUM); alternate DVE/ACT so
                # the 8 copies pipeline ~2x
                for n in range(8):
                    dst = eT_sb[:, n * 512:(n + 1) * 512]
                    if n % 2 == 0:
                        nc.vector.tensor_copy(dst, psbank[n])
                    else:
                        nc.scalar.copy(dst, psbank[n])
                # F gather: column block [512g, 512(g+1)) of e^T is exactly
                # scan group g's (c3, s, b) panel -> straight [9,512] copy
                for g in range(NG):
                    nc.sync.dma_start(f_raw[g * T:(g + 1) * T],
                                      eT_sb[:, g * 512:(g + 1) * 512])

            # F_all[(g,j), (c3, s, b)] = exp(e - sigma + fcb) elementwise
            f_all = fpool.tile([P_SCAN, C3 * L * NB], dt.float32)
            nc.scalar.activation(f_all, f_raw,
                                 mybir.ActivationFunctionType.Exp, bias=biasF_sb)
            # chunk-0 s=0 slots: alpha_0 = exp(start_j + fcb_j + e_0[b,j])
            nc.scalar.activation(
                f_all[0:T, 0:NB], f_raw[0:T, 0:NB],
                mybir.ActivationFunctionType.Exp, bias=bias0_sb)
            f_v = f_all.rearrange("p (c3 s b) -> p c3 s b", c3=C3, s=L)

            # ---- scan: chunk c = 8g + c3, 8 steps, two independent
            # half-chains (c3 0-3 | 4-7) that interleave on PE/DVE ----
            def fslice(s, h):
                return f_v[:, 4 * h:4 * h + 4, s, :].unsqueeze(-1).broadcast_to(
                    [P_SCAN, 4, NB, T])

            with tc.tile_pool(name="psq", bufs=4, space="PSUM") as psqpool:
                qf = []
                for h in range(2):
                    q = qpool.tile([P_SCAN, 4, NB, T], dt.bfloat16, tag=f"q{h}",
                                   name=f"q{h}")
                    ep = epat_sb[:, h * HALF:(h + 1) * HALF]
                    nc.vector.tensor_mul(
                        q, ep.rearrange("p (c3 b i) -> p c3 b i", c3=4, b=NB),
                        fslice(0, h))
                    for s in range(1, L):
                        psq = psqpool.tile([P_SCAN, HALF], dt.float32, tag="psq",
                                           name="psq")
                        nc.tensor.matmul(psq, lhsE_sb,
                                         q.rearrange("p c3 b i -> p (c3 b i)"),
                                         start=True, stop=True)
                        qn = qpool.tile([P_SCAN, 4, NB, T], dt.bfloat16,
                                        tag=f"q{h}", name=f"qn{h}")
                        nc.vector.tensor_mul(
                            qn, psq.rearrange("p (c3 b i) -> p c3 b i",
                                              c3=4, b=NB),
                            fslice(s, h))
                        q = qn
                    # back to f32 for the host
                    qff = fpool.tile([P_SCAN, HALF], dt.float32, tag=f"qf{h}",
                                     name=f"qf{h}")
                    nc.vector.tensor_copy(qff, q.rearrange("p c3 b i -> p (c3 b i)"))
                    qf.append(qff)
                for h in range(2):
                    nc.gpsimd.dma_start(q_out[:, h * HALF:(h + 1) * HALF], qf[h])
            nc.gpsimd.dma_start(eT_out[:, :], eT_sb)

    nc.compile()
    return nc


def _get_nc():
    if "nc" not in _cached:
        _cached["nc"] = _build_nc()
    return _cached["nc"]


def _host_prep(hidden_states, fc_w, fc_b, start_transitions, transitions):
    """Build the 8 per-core input maps."""
    import ml_dtypes
    np_mdt = ml_dtypes.bfloat16 if MOVING_DTYPE == "bf16" else np.float32

    E = np.exp(transitions.astype(np.float64)).astype(np.float32)     # [T,T]
    # epat[(g,j),(c3,b,i)] = E[i,j], except chunk 0 (g=0, c3=0) slots = 1
    epat = np.tile(E.T[None, :, None, None, :], (NG, 1, C3, NB, 1))   # [g,j,c3,b,i]
    epat[0, :, 0, :, :] = 1.0
    epat = np.ascontiguousarray(epat.reshape(P_SCAN, NFREE), dtype=np.float32)
    # lhsE = blockdiag(E) x8: lhsT[(g,k),(g,j)] = E[k,j]  (bf16 scan matmul)
    lhsE = np.zeros((P_SCAN, P_SCAN), dtype=ml_dtypes.bfloat16)
    for g in range(NG):
        lhsE[g * T:(g + 1) * T, g * T:(g + 1) * T] = E.astype(ml_dtypes.bfloat16)
    fcwT = np.ascontiguousarray(fc_w.T.astype(np_mdt))                # [H,T]
    biasF = np.ascontiguousarray(
        np.tile(fc_b - SIGMA, NG).reshape(P_SCAN, 1), dtype=np.float32)
    bias0 = np.ascontiguousarray(
        (start_transitions + fc_b).reshape(T, 1), dtype=np.float32)

    in_maps = []
    for cid in range(NCORES):
        hc = hidden_states[cid * NB:(cid + 1) * NB]                   # [NB,S,H]
        # t-major token order: col = t*NB + b
        hc = hc.transpose(1, 0, 2).reshape(NTOK, H)
        hTc = np.ascontiguousarray(hc.T.astype(np_mdt))               # [H,4096]
        in_maps.append({
            "hT": hTc, "fcwT": fcwT, "lhsE": lhsE, "epat": epat,
            "biasF": biasF, "bias0": bias0,
        })
    return in_maps


def _host_finish(results, labels, fc_b, start_transitions,
                 end_transitions, transitions):
    """Numerator + chunk-matrix combine, all in f64."""
    labels = labels.astype(np.int64)
    start = start_transitions.astype(np.float64)
    end = end_transitions.astype(np.float64)
    trans = transitions.astype(np.float64)

    # reassemble e [B, S, T] from per-core e^T [9, 4096] (+ fc_b)
    # token order is t-major: col = t*NB + b
    e = np.empty((B, S, T), dtype=np.float64)
    for cid in range(NCORES):
        eT = results[cid]["eT_out"].astype(np.float64)    # [9, 4096]
        e[cid * NB:(cid + 1) * NB] = eT.T.reshape(S, NB, T).transpose(1, 0, 2)
    e += fc_b.astype(np.float64)

    # numerator (mask all-ones fast path)
    emit = np.take_along_axis(e, labels[..., None], axis=-1)[..., 0]
    num = start[labels[:, 0]] + emit[:, 0]
    num = num + (trans[labels[:, :-1], labels[:, 1:]] + emit[:, 1:]).sum(1)
    num = num + end[labels[:, -1]]

    # denominator: combine chunk matrices
    # chunk c = 8g + c3;  Q[(g,j), (c3,b,i)] = P_c[i, j]
    den = np.empty(B)
    for cid in range(NCORES):
        Q = results[cid]["q_out"].astype(np.float64)      # [72, 576]
        Q = Q.reshape(NG, T, C3, NB, T)                   # [g, j, c3, b, i]
        for b in range(NB):
            alpha = Q[0, :, 0, b, 0].copy()  # P_0[0,:] (rows of P_0 all equal)
            corr = 0.0
            for c in range(1, C):
                g, c3 = c // C3, c % C3
                Pc = Q[g, :, c3, b, :].T                  # P_c[i, j] rows i
                alpha = alpha @ Pc
                m = alpha.max()
                alpha /= m
                corr += np.log(m)
            den[cid * NB + b] = np.log((alpha * np.exp(end)).sum()) + corr \
                + (S - 1) * SIGMA
    return np.float32(-(num - den).sum())


def kernel(**inputs):
    hidden_states = np.asarray(inputs["hidden_states"], dtype=np.float32)
    attention_mask = np.asarray(inputs["attention_mask"])
    labels = np.asarray(inputs["labels"])
    fc_w = np.asarray(inputs["fc_w"], dtype=np.float32)
    fc_b = np.asarray(inputs["fc_b"], dtype=np.float32)
    start_transitions = np.asarray(inputs["start_transitions"], dtype=np.float32)
    end_transitions = np.asarray(inputs["end_transitions"], dtype=np.float32)
    transitions = np.asarray(inputs["transitions"], dtype=np.float32)

    if (hidden_states.shape != (B, S, H)) or not np.all(attention_mask != 0):
        return _reference_host(hidden_states, attention_mask, labels, fc_w,
                               fc_b, start_transitions, end_transitions,
                               transitions)

    from concourse.bass_utils import run_bass_kernel_spmd
    nc = _get_nc()
    in_maps = _host_prep(hidden_states, fc_w, fc_b, start_transitions,
                         transitions)
    res = run_bass_kernel_spmd(nc, in_maps, core_ids=list(range(NCORES)))
    _cached["last_res"] = res
    return _host_finish(res.results, labels, fc_b, start_transitions,
                        end_transitions, transitions)


# revision 39
# speedup vs baseline: 1.2540x; 1.0567x over previous
"""BertCrf loss kernel for Trainium2 (8 NeuronCores, SPMD data-parallel).

Strategy
--------
Shapes: B=64, S=512, H=768, T=9 tags.  Loss = -sum_b(num_b - den_b).

The only heavy data is hidden_states [64,512,768] f32 (100 MB) -> the kernel
is memory-bound on streaming it once.  Each of the 8 cores takes 8 sequences.

Phase 1 (device, DMA-bound): emissions e^T [9, 4096] = fc_w @ h^T per core,
streamed from a host-pre-transposed hT [768, 4096] so the contraction dim
lands on partitions.  e^T (without fc_b; host adds it) is DMA'd back to the
host (147 KB/core) for the numerator.

Phase 2 (device): the CRF log-partition recurrence
  alpha_t[j] = logsumexp_k(alpha_{t-1,k} + trans[k,j]) + e_t[j]
is associative in the (log,+) semiring.  In linear space each step is
  P <- P @ (E * f_t[None,:]),  E = exp(trans), f_t = exp(e_t + fc_b - sigma),
so each 32-step chunk's product matrix is computed independently ->
8 seqs x 16 chunks = 128 independent 9x9 matrix chains.  These are packed
as 8 block-diagonal groups on the TensorEngine: one [72,72]x[72,144]
matmul + one [72,144] VectorE scale per step computes ALL 128 chunks.
The per-pair scale factors F are built from e^T by a small on-chip
shuffle DMA + one Exp activation.  The constant shift sigma keeps fp32 in
range (chunk log-range ~ +-45; fp32 overflows at 88).

Host (cheap, exact f64): numerator from labels + e^T; combine the 16 chunk
matrices per sequence (tiny 9x9 matvecs) with renormalization; final
logsumexp with end_transitions.  A full numpy fallback handles any
non-all-ones attention mask (the benchmark's mask is always ones).

Scan packing (hardcoded):
  pair (c, b): chunk c in [0,16), local seq b in [0,8)
  group g = c//2, c2 = c%2     -> partitions (g, j) = 8*9 = 72
  free index (c2, b, i)        -> 2*8*9 = 144 columns
  Q[(g,j), (c2,b,i)] = P_{c,b}[i, j]   (state, transposed per pair)
  step: matmul out[(g,j),n] = sum_k blockdiag(E)[(g,k),(g,j)] Q[(g,k),n],
        then Q <- out * F[(g,j),(c2,b,s)] broadcast over i.
  step s=0 is pure elementwise: Q_1 = Epat * F[...,0] where
  Epat[(g,j),(c2,b,i)] = 1.0 if chunk==0 else E[i,j]  (chunk 0's s=0 slot
  holds exp(start_j + e_0[j] + fc_b[j]) -> rows of P_0 all equal alpha_0).
"""

import numpy as np

# ---- problem constants (hardcoded per the task contract) ----
B, S, H, T = 64, 512, 768, 9
NCORES = 8
NB = B // NCORES          # 8 local sequences per core
NTOK = NB * S             # 4096 tokens per core
L = 8                     # chunk length (timesteps per chunk)
C = S // L                # 64 chunks
NG = 8                    # partition groups; chunk c = 8g + c3
C3 = 8                    # chunks per group
P_SCAN = NG * T           # 72 scan partitions
NFREE = C3 * NB * T       # 576 scan free columns (c3, b, i)
SIGMA = 0.8               # linear-space shift (range control)
KT = H // 128             # 6 contraction tiles

# token order is t-major: column index = t*NB + b.  Then PSUM bank g of the
# emissions matmul (columns [512g, 512g+512)) is exactly scan group g's
# (c3, s, b) panel: col = 512g + 64*c3 + 8*s + b.

MOVING_DTYPE = "bf16"     # "f32" | "f32r" | "bf16"  (hidden/fc_w matmul dtype)

_cached = {}


def _np_logsumexp(x, axis):
    m = np.max(x, axis=axis, keepdims=True)
    return (m + np.log(np.sum(np.exp(x - m), axis=axis, keepdims=True))).squeeze(axis)


def _reference_host(hidden_states, attention_mask, labels, fc_w, fc_b,
                    start_transitions, end_transitions, transitions):
    """Exact numpy port of the reference (f64) - fallback for unusual inputs."""
    e = (hidden_states.astype(np.float64) @ fc_w.T.astype(np.float64)) + fc_b
    mask = attention_mask.astype(bool)
    maskf = mask.astype(np.float64)
    labels = labels.astype(np.int64)
    b_idx = np.arange(e.shape[0])

    emit = np.take_along_axis(e, labels[..., None], axis=-1)[..., 0]
    trans_sc = transitions[labels[:, :-1], labels[:, 1:]].astype(np.float64)
    num = start_transitions[labels[:, 0]].astype(np.float64) + emit[:, 0]
    num = num + ((trans_sc + emit[:, 1:]) * maskf[:, 1:]).sum(1)
    last_idx = mask.astype(np.int64).sum(1) - 1
    num = num + end_transitions[labels[b_idx, last_idx]]

    alpha = start_transitions[None, :].astype(np.float64) + e[:, 0]
    for t in range(1, e.shape[1]):
        nxt = _np_logsumexp(alpha[:, :, None] + transitions[None].astype(np.float64)
                            + e[:, t][:, None, :], axis=1)
        alpha = np.where(mask[:, t][:, None], nxt, alpha)
    den = _np_logsumexp(alpha + end_transitions[None, :].astype(np.float64), axis=1)
    return np.float32(-(num - den).sum())


def _build_nc():
    """Build the per-core Bass program (same program on all 8 cores)."""
    import concourse.bacc as bacc
    import concourse.mybir as mybir
    import concourse.tile as tile

    dt = mybir.dt
    mdt = {"f32": dt.float32, "f32r": dt.float32, "bf16": dt.bfloat16}[MOVING_DTYPE]

    # Bacc (not raw Bass): its compile() pass legalizes multi-wait sync_info
    # into what this walrus build's per-instruction wait slots accept.
    nc = bacc.Bacc("TRN2", target_bir_lowering=False, debug=False)

    hT = nc.dram_tensor("hT", [H, NTOK], mdt, kind="ExternalInput")
    fcwT = nc.dram_tensor("fcwT", [H, T], mdt, kind="ExternalInput")
    lhsE = nc.dram_tensor("lhsE", [P_SCAN, P_SCAN], dt.bfloat16, kind="ExternalInput")
    epat = nc.dram_tensor("epat", [P_SCAN, NFREE], dt.float32, kind="ExternalInput")
    biasF = nc.dram_tensor("biasF", [P_SCAN, 1], dt.float32, kind="ExternalInput")
    bias0 = nc.dram_tensor("bias0", [T, 1], dt.float32, kind="ExternalInput")
    eT_out = nc.dram_tensor("eT_out", [T, NTOK], dt.float32, kind="ExternalOutput")
    q_out = nc.dram_tensor("q_out", [P_SCAN, NFREE], dt.float32,
                           kind="ExternalOutput")

    f32r = dt.float32r
    HALF = NFREE // 2          # 288 free columns per scan half-chain

    with tile.TileContext(nc) as tc:
        with (
            tc.tile_pool(name="const", bufs=1) as cpool,
            tc.tile_pool(name="hbuf", bufs=1) as hpool,
            tc.tile_pool(name="fbuf", bufs=1) as fpool,
            tc.tile_pool(name="scan", bufs=2) as qpool,
        ):
            # ---- phase 1 loads: 6 full-row 1MB DMAs over THREE parallel DMA
            # paths (sync HWDGE, ACT HWDGE, gpsimd SWDGE) - each path's
            # descriptor generation is the throughput limit (~8 us/MB/ring)
            hT_r = hT.rearrange("(kt p) n -> kt p n", p=128)
            fcw_sb = cpool.tile([128, KT, T], mdt)
            nc.gpsimd.dma_start(fcw_sb, fcwT.rearrange("(kt p) m -> p kt m", p=128))
            path = {0: nc.sync, 3: nc.sync, 1: nc.scalar, 4: nc.scalar,
                    2: nc.gpsimd, 5: nc.gpsimd}
            htiles = [None] * KT
            for kt in [0, 1, 2, 3, 4, 5]:
                ht = hpool.tile([128, NTOK], mdt, tag=f"ht{kt}", name=f"ht{kt}")
                path[kt].dma_start(ht, hT_r[kt])
                htiles[kt] = ht
            # remaining constants (needed only by the scan) follow on SWDGE
            lhsE_sb = cpool.tile([P_SCAN, P_SCAN], dt.bfloat16)
            nc.gpsimd.dma_start(lhsE_sb, lhsE[:, :])
            epat_sb = cpool.tile([P_SCAN, NFREE], dt.float32)
            nc.gpsimd.dma_start(epat_sb, epat[:, :])
            biasF_sb = cpool.tile([P_SCAN, 1], dt.float32)
            nc.gpsimd.dma_start(biasF_sb, biasF[:, :])
            bias0_sb = cpool.tile([T, 1], dt.float32)
            nc.gpsimd.dma_start(bias0_sb, bias0[:, :])

            f_raw = fpool.tile([P_SCAN, C3 * L * NB], dt.float32)
            eT_sb = fpool.tile([T, NTOK], dt.float32)
            with tc.tile_pool(name="psum1", bufs=1, space="PSUM") as pspool:
                psbank = [pspool.tile([T, 512], dt.float32, tag=f"psb{i}",
                                      name=f"psb{i}")
                          for i in range(8)]
                for kt in range(KT):
                    for n in range(8):
                        lw = fcw_sb[:, kt, :]
                        rh = htiles[kt][:, n * 512:(n + 1) * 512]
                        if MOVING_DTYPE == "f32r":
                            lw = lw.bitcast(f32r)
                            rh = rh.bitcast(f32r)
                        nc.tensor.matmul(psbank[n], lw, rh,
                                         start=(kt == 0), stop=(kt == KT - 1))
                # PSUM -> SBUF (DMA cannot read PSUM); alternate DVE/ACT so
                # the 8 copies pipeline ~2x, each bank's F gather follows
                # immediately on the sync ring (idle once hT is done).
                # Column block [512g, 512(g+1)) of e^T is exactly scan group
                # g's (c3, s, b) panel -> straight [9,512] copy.
                for g in range(NG):
                    dst = eT_sb[:, g * 512:(g + 1) * 512]
                    if g % 2 == 0:
                        nc.vector.tensor_copy(dst, psbank[g])
                    else:
                        nc.scalar.copy(dst, psbank[g])
                    nc.sync.dma_start(f_raw[g * T:(g + 1) * T], dst)

            # F_all[(g,j), (c3, s, b)] = exp(e - sigma + fcb) elementwise
            f_all = fpool.tile([P_SCAN, C3 * L * NB], dt.float32)
            nc.scalar.activation(f_all, f_raw,
                                 mybir.ActivationFunctionType.Exp, bias=biasF_sb)
            # chunk-0 s=0 slots: alpha_0 = exp(start_j + fcb_j + e_0[b,j])
            nc.scalar.activation(
                f_all[0:T, 0:NB], f_raw[0:T, 0:NB],
                mybir.ActivationFunctionType.Exp, bias=bias0_sb)
            f_v = f_all.rearrange("p (c3 s b) -> p c3 s b", c3=C3, s=L)

            # ---- scan: chunk c = 8g + c3, 8 steps, two independent
            # half-chains (c3 0-3 | 4-7) that interleave on PE/DVE ----
            def fslice(s, h):
                return f_v[:, 4 * h:4 * h + 4, s, :].unsqueeze(-1).broadcast_to(
                    [P_SCAN, 4, NB, T])

            # interleave the two chains' emission: engines run their program
            # IN ORDER, so A1,B1,A2,B2,... lets chain B's matmul fill the PE
            # while chain A's scale runs on the DVE (and vice versa)
            with tc.tile_pool(name="psq", bufs=4, space="PSUM") as psqpool:
                qcur = []
                for h in range(2):
                    q = qpool.tile([P_SCAN, 4, NB, T], dt.bfloat16, tag=f"q{h}",
                                   name=f"q{h}")
                    ep = epat_sb[:, h * HALF:(h + 1) * HALF]
                    nc.vector.tensor_mul(
                        q, ep.rearrange("p (c3 b i) -> p c3 b i", c3=4, b=NB),
                        fslice(0, h))
                    qcur.append(q)
                for s in range(1, L):
                    for h in range(2):
                        psq = psqpool.tile([P_SCAN, HALF], dt.float32, tag="psq",
                                           name="psq")
                        nc.tensor.matmul(
                            psq, lhsE_sb,
                            qcur[h].rearrange("p c3 b i -> p (c3 b i)"),
                            start=True, stop=True)
                        qn = qpool.tile([P_SCAN, 4, NB, T], dt.bfloat16,
                                        tag=f"q{h}", name=f"qn{h}")
                        nc.vector.tensor_mul(
                            qn, psq.rearrange("p (c3 b i) -> p c3 b i",
                                              c3=4, b=NB),
                            fslice(s, h))
                        qcur[h] = qn
                for h in range(2):
                    # back to f32 for the host
                    qff = fpool.tile([P_SCAN, HALF], dt.float32, tag=f"qf{h}",
                                     name=f"qf{h}")
                    nc.vector.tensor_copy(
                        qff, qcur[h].rearrange("p c3 b i -> p (c3 b i)"))
                    nc.gpsimd.dma_start(q_out[:, h * HALF:(h + 1) * HALF], qff)
            nc.gpsimd.dma_start(eT_out[:, :], eT_sb)

    nc.compile()
    return nc


def _get_nc():
    if "nc" not in _cached:
        _cached["nc"] = _build_nc()
    return _cached["nc"]


def _host_prep(hidden_states, fc_w, fc_b, start_transitions, transitions):
    """Build the 8 per-core input maps."""
    import ml_dtypes
    np_mdt = ml_dtypes.bfloat16 if MOVING_DTYPE == "bf16" else np.float32

    E = np.exp(transitions.astype(np.float64)).astype(np.float32)     # [T,T]
    # epat[(g,j),(c3,b,i)] = E[i,j], except chunk 0 (g=0, c3=0) slots = 1
    epat = np.tile(E.T[None, :, None, None, :], (NG, 1, C3, NB, 1))   # [g,j,c3,b,i]
    epat[0, :, 0, :, :] = 1.0
    epat = np.ascontiguousarray(epat.reshape(P_SCAN, NFREE), dtype=np.float32)
    # lhsE = blockdiag(E) x8: lhsT[(g,k),(g,j)] = E[k,j]  (bf16 scan matmul)
    lhsE = np.zeros((P_SCAN, P_SCAN), dtype=ml_dtypes.bfloat16)
    for g in range(NG):
        lhsE[g * T:(g + 1) * T, g * T:(g + 1) * T] = E.astype(ml_dtypes.bfloat16)
    fcwT = np.ascontiguousarray(fc_w.T.astype(np_mdt))                # [H,T]
    biasF = np.ascontiguousarray(
        np.tile(fc_b - SIGMA, NG).reshape(P_SCAN, 1), dtype=np.float32)
    bias0 = np.ascontiguousarray(
        (start_transitions + fc_b).reshape(T, 1), dtype=np.float32)

    in_maps = []
    for cid in range(NCORES):
        hc = hidden_states[cid * NB:(cid + 1) * NB]                   # [NB,S,H]
        # t-major token order: col = t*NB + b
        hc = hc.transpose(1, 0, 2).reshape(NTOK, H)
        hTc = np.ascontiguousarray(hc.T.astype(np_mdt))               # [H,4096]
        in_maps.append({
            "hT": hTc, "fcwT": fcwT, "lhsE": lhsE, "epat": epat,
            "biasF": biasF, "bias0": bias0,
        })
    return in_maps


def _host_finish(results, labels, fc_b, start_transitions,
                 end_transitions, transitions):
    """Numerator + chunk-matrix combine, all in f64."""
    labels = labels.astype(np.int64)
    start = start_transitions.astype(np.float64)
    end = end_transitions.astype(np.float64)
    trans = transitions.astype(np.float64)

    # reassemble e [B, S, T] from per-core e^T [9, 4096] (+ fc_b)
    # token order is t-major: col = t*NB + b
    e = np.empty((B, S, T), dtype=np.float64)
    for cid in range(NCORES):
        eT = results[cid]["eT_out"].astype(np.float64)    # [9, 4096]
        e[cid * NB:(cid + 1) * NB] = eT.T.reshape(S, NB, T).transpose(1, 0, 2)
    e += fc_b.astype(np.float64)

    # numerator (mask all-ones fast path)
    emit = np.take_along_axis(e, labels[..., None], axis=-1)[..., 0]
    num = start[labels[:, 0]] + emit[:, 0]
    num = num + (trans[labels[:, :-1], labels[:, 1:]] + emit[:, 1:]).sum(1)
    num = num + end[labels[:, -1]]

    # denominator: combine chunk matrices
    # chunk c = 8g + c3;  Q[(g,j), (c3,b,i)] = P_c[i, j]
    den = np.empty(B)
    for cid in range(NCORES):
        Q = results[cid]["q_out"].astype(np.float64)      # [72, 576]
        Q = Q.reshape(NG, T, C3, NB, T)                   # [g, j, c3, b, i]
        for b in range(NB):
            alpha = Q[0, :, 0, b, 0].copy()  # P_0[0,:] (rows of P_0 all equal)
            corr = 0.0
            for c in range(1, C):
                g, c3 = c // C3, c % C3
                Pc = Q[g, :, c3, b, :].T                  # P_c[i, j] rows i
                alpha = alpha @ Pc
                m = alpha.max()
                alpha /= m
                corr += np.log(m)
            den[cid * NB + b] = np.log((alpha * np.exp(end)).sum()) + corr \
                + (S - 1) * SIGMA
    return np.float32(-(num - den).sum())


def kernel(**inputs):
    hidden_states = np.asarray(inputs["hidden_states"], dtype=np.float32)
    attention_mask = np.asarray(inputs["attention_mask"])
    labels = np.asarray(inputs["labels"])
    fc_w = np.asarray(inputs["fc_w"], dtype=np.float32)
    fc_b = np.asarray(inputs["fc_b"], dtype=np.float32)
    start_transitions = np.asarray(inputs["start_transitions"], dtype=np.float32)
    end_transitions = np.asarray(inputs["end_transitions"], dtype=np.float32)
    transitions = np.asarray(inputs["transitions"], dtype=np.float32)

    if (hidden_states.shape != (B, S, H)) or not np.all(attention_mask != 0):
        return _reference_host(hidden_states, attention_mask, labels, fc_w,
                               fc_b, start_transitions, end_transitions,
                               transitions)

    from concourse.bass_utils import run_bass_kernel_spmd
    nc = _get_nc()
    in_maps = _host_prep(hidden_states, fc_w, fc_b, start_transitions,
                         transitions)
    res = run_bass_kernel_spmd(nc, in_maps, core_ids=list(range(NCORES)))
    _cached["last_res"] = res
    return _host_finish(res.results, labels, fc_b, start_transitions,
                        end_transitions, transitions)


# revision 40
# speedup vs baseline: 1.3290x; 1.0598x over previous
"""BertCrf loss kernel for Trainium2 (8 NeuronCores, SPMD data-parallel).

Strategy
--------
Shapes: B=64, S=512, H=768, T=9 tags.  Loss = -sum_b(num_b - den_b).

The only heavy data is hidden_states [64,512,768] f32 (100 MB) -> the kernel
is memory-bound on streaming it once.  Each of the 8 cores takes 8 sequences.

Phase 1 (device, DMA-bound): emissions e^T [9, 4096] = fc_w @ h^T per core,
streamed from a host-pre-transposed hT [768, 4096] so the contraction dim
lands on partitions.  e^T (without fc_b; host adds it) is DMA'd back to the
host (147 KB/core) for the numerator.

Phase 2 (device): the CRF log-partition recurrence
  alpha_t[j] = logsumexp_k(alpha_{t-1,k} + trans[k,j]) + e_t[j]
is associative in the (log,+) semiring.  In linear space each step is
  P <- P @ (E * f_t[None,:]),  E = exp(trans), f_t = exp(e_t + fc_b - sigma),
so each 32-step chunk's product matrix is computed independently ->
8 seqs x 16 chunks = 128 independent 9x9 matrix chains.  These are packed
as 8 block-diagonal groups on the TensorEngine: one [72,72]x[72,144]
matmul + one [72,144] VectorE scale per step computes ALL 128 chunks.
The per-pair scale factors F are built from e^T by a small on-chip
shuffle DMA + one Exp activation.  The constant shift sigma keeps fp32 in
range (chunk log-range ~ +-45; fp32 overflows at 88).

Host (cheap, exact f64): numerator from labels + e^T; combine the 16 chunk
matrices per sequence (tiny 9x9 matvecs) with renormalization; final
logsumexp with end_transitions.  A full numpy fallback handles any
non-all-ones attention mask (the benchmark's mask is always ones).

Scan packing (hardcoded):
  pair (c, b): chunk c in [0,16), local seq b in [0,8)
  group g = c//2, c2 = c%2     -> partitions (g, j) = 8*9 = 72
  free index (c2, b, i)        -> 2*8*9 = 144 columns
  Q[(g,j), (c2,b,i)] = P_{c,b}[i, j]   (state, transposed per pair)
  step: matmul out[(g,j),n] = sum_k blockdiag(E)[(g,k),(g,j)] Q[(g,k),n],
        then Q <- out * F[(g,j),(c2,b,s)] broadcast over i.
  step s=0 is pure elementwise: Q_1 = Epat * F[...,0] where
  Epat[(g,j),(c2,b,i)] = 1.0 if chunk==0 else E[i,j]  (chunk 0's s=0 slot
  holds exp(start_j + e_0[j] + fc_b[j]) -> rows of P_0 all equal alpha_0).
"""

import numpy as np

# ---- problem constants (hardcoded per the task contract) ----
B, S, H, T = 64, 512, 768, 9
NCORES = 8
NB = B // NCORES          # 8 local sequences per core
NTOK = NB * S             # 4096 tokens per core
L = 8                     # chunk length (timesteps per chunk)
C = S // L                # 64 chunks
NG = 8                    # partition groups; chunk c = 8g + c3
C3 = 8                    # chunks per group
P_SCAN = NG * T           # 72 scan partitions
NFREE = C3 * NB * T       # 576 scan free columns (c3, b, i)
SIGMA = 0.8               # linear-space shift (range control)
KT = H // 128             # 6 contraction tiles

# token order is t-major: column index = t*NB + b.  Then PSUM bank g of the
# emissions matmul (columns [512g, 512g+512)) is exactly scan group g's
# (c3, s, b) panel: col = 512g + 64*c3 + 8*s + b.

MOVING_DTYPE = "bf16"     # "f32" | "f32r" | "bf16"  (hidden/fc_w matmul dtype)

_cached = {}


def _np_logsumexp(x, axis):
    m = np.max(x, axis=axis, keepdims=True)
    return (m + np.log(np.sum(np.exp(x - m), axis=axis, keepdims=True))).squeeze(axis)


def _reference_host(hidden_states, attention_mask, labels, fc_w, fc_b,
                    start_transitions, end_transitions, transitions):
    """Exact numpy port of the reference (f64) - fallback for unusual inputs."""
    e = (hidden_states.astype(np.float64) @ fc_w.T.astype(np.float64)) + fc_b
    mask = attention_mask.astype(bool)
    maskf = mask.astype(np.float64)
    labels = labels.astype(np.int64)
    b_idx = np.arange(e.shape[0])

    emit = np.take_along_axis(e, labels[..., None], axis=-1)[..., 0]
    trans_sc = transitions[labels[:, :-1], labels[:, 1:]].astype(np.float64)
    num = start_transitions[labels[:, 0]].astype(np.float64) + emit[:, 0]
    num = num + ((trans_sc + emit[:, 1:]) * maskf[:, 1:]).sum(1)
    last_idx = mask.astype(np.int64).sum(1) - 1
    num = num + end_transitions[labels[b_idx, last_idx]]

    alpha = start_transitions[None, :].astype(np.float64) + e[:, 0]
    for t in range(1, e.shape[1]):
        nxt = _np_logsumexp(alpha[:, :, None] + transitions[None].astype(np.float64)
                            + e[:, t][:, None, :], axis=1)
        alpha = np.where(mask[:, t][:, None], nxt, alpha)
    den = _np_logsumexp(alpha + end_transitions[None, :].astype(np.float64), axis=1)
    return np.float32(-(num - den).sum())


def _build_nc():
    """Build the per-core Bass program (same program on all 8 cores)."""
    import concourse.bacc as bacc
    import concourse.mybir as mybir
    import concourse.tile as tile

    dt = mybir.dt
    mdt = {"f32": dt.float32, "f32r": dt.float32, "bf16": dt.bfloat16}[MOVING_DTYPE]

    # Bacc (not raw Bass): its compile() pass legalizes multi-wait sync_info
    # into what this walrus build's per-instruction wait slots accept.
    nc = bacc.Bacc("TRN2", target_bir_lowering=False, debug=False)

    hT = nc.dram_tensor("hT", [H, NTOK], mdt, kind="ExternalInput")
    fcwT = nc.dram_tensor("fcwT", [H, T], mdt, kind="ExternalInput")
    lhsE = nc.dram_tensor("lhsE", [P_SCAN, P_SCAN], dt.bfloat16, kind="ExternalInput")
    epat = nc.dram_tensor("epat", [P_SCAN, NFREE], dt.float32, kind="ExternalInput")
    biasF = nc.dram_tensor("biasF", [P_SCAN, 1], dt.float32, kind="ExternalInput")
    bias0 = nc.dram_tensor("bias0", [T, 1], dt.float32, kind="ExternalInput")
    eT_out = nc.dram_tensor("eT_out", [T, NTOK], dt.float32, kind="ExternalOutput")
    q_out = nc.dram_tensor("q_out", [P_SCAN, NFREE], dt.float32,
                           kind="ExternalOutput")

    f32r = dt.float32r
    HALF = NFREE // 2          # 288 free columns per scan half-chain

    with tile.TileContext(nc) as tc:
        with (
            tc.tile_pool(name="const", bufs=1) as cpool,
            tc.tile_pool(name="hbuf", bufs=1) as hpool,
            tc.tile_pool(name="fbuf", bufs=1) as fpool,
            tc.tile_pool(name="scan", bufs=2) as qpool,
        ):
            # ---- phase 1 loads: 12 half-row 512KB DMAs on the two HWDGE
            # rings, HALF-MAJOR order: columns [0,2048) land first so banks
            # 0-3 finish (and their copies + F gathers run) while columns
            # [2048,4096) are still streaming ----
            hT_r = hT.rearrange("(kt p) n -> kt p n", p=128)
            fcw_sb = cpool.tile([128, KT, T], mdt)
            nc.gpsimd.dma_start(fcw_sb, fcwT.rearrange("(kt p) m -> p kt m", p=128))
            lhsE_sb = cpool.tile([P_SCAN, P_SCAN], dt.bfloat16)
            nc.gpsimd.dma_start(lhsE_sb, lhsE[:, :])
            epat_sb = cpool.tile([P_SCAN, NFREE], dt.float32)
            nc.gpsimd.dma_start(epat_sb, epat[:, :])
            biasF_sb = cpool.tile([P_SCAN, 1], dt.float32)
            nc.gpsimd.dma_start(biasF_sb, biasF[:, :])
            bias0_sb = cpool.tile([T, 1], dt.float32)
            nc.gpsimd.dma_start(bias0_sb, bias0[:, :])

            HC = NTOK // 2
            htiles = [[None] * 2 for _ in range(KT)]
            for h in range(2):
                for kt in range(KT):
                    ht = hpool.tile([128, HC], mdt, tag=f"ht{kt}_{h}",
                                    name=f"ht{kt}_{h}")
                    eng = nc.sync if kt % 2 == 0 else nc.scalar
                    eng.dma_start(ht, hT_r[kt, :, h * HC:(h + 1) * HC])
                    htiles[kt][h] = ht

            f_raw = fpool.tile([P_SCAN, C3 * L * NB], dt.float32)
            eT_sb = fpool.tile([T, NTOK], dt.float32)
            with tc.tile_pool(name="psum1", bufs=1, space="PSUM") as pspool:
                psbank = [pspool.tile([T, 512], dt.float32, tag=f"psb{i}",
                                      name=f"psb{i}")
                          for i in range(8)]
                for h in range(2):
                    for kt in range(KT):
                        for nn in range(4):
                            n = 4 * h + nn
                            lw = fcw_sb[:, kt, :]
                            rh = htiles[kt][h][:, nn * 512:(nn + 1) * 512]
                            if MOVING_DTYPE == "f32r":
                                lw = lw.bitcast(f32r)
                                rh = rh.bitcast(f32r)
                            nc.tensor.matmul(psbank[n], lw, rh,
                                             start=(kt == 0), stop=(kt == KT - 1))
                    # banks of this half are done: PSUM->SBUF copy (DMA can't
                    # read PSUM; alternate DVE/ACT), then the F gather for
                    # scan group g on the SWDGE queue (rings are still
                    # streaming hT).  Column block [512g, 512(g+1)) of e^T is
                    # exactly group g's (c3, s, b) panel.
                    for nn in range(4):
                        g = 4 * h + nn
                        dst = eT_sb[:, g * 512:(g + 1) * 512]
                        if g % 2 == 0:
                            nc.vector.tensor_copy(dst, psbank[g])
                        else:
                            nc.scalar.copy(dst, psbank[g])
                        nc.gpsimd.dma_start(f_raw[g * T:(g + 1) * T], dst)

            # F_all[(g,j), (c3, s, b)] = exp(e - sigma + fcb) elementwise
            f_all = fpool.tile([P_SCAN, C3 * L * NB], dt.float32)
            nc.scalar.activation(f_all, f_raw,
                                 mybir.ActivationFunctionType.Exp, bias=biasF_sb)
            # chunk-0 s=0 slots: alpha_0 = exp(start_j + fcb_j + e_0[b,j])
            nc.scalar.activation(
                f_all[0:T, 0:NB], f_raw[0:T, 0:NB],
                mybir.ActivationFunctionType.Exp, bias=bias0_sb)
            f_v = f_all.rearrange("p (c3 s b) -> p c3 s b", c3=C3, s=L)

            # ---- scan: chunk c = 8g + c3, 8 steps, two independent
            # half-chains (c3 0-3 | 4-7) that interleave on PE/DVE ----
            def fslice(s, h):
                return f_v[:, 4 * h:4 * h + 4, s, :].unsqueeze(-1).broadcast_to(
                    [P_SCAN, 4, NB, T])

            # interleave the two chains' emission: engines run their program
            # IN ORDER, so A1,B1,A2,B2,... lets chain B's matmul fill the PE
            # while chain A's scale runs on the DVE (and vice versa)
            with tc.tile_pool(name="psq", bufs=4, space="PSUM") as psqpool:
                qcur = []
                for h in range(2):
                    q = qpool.tile([P_SCAN, 4, NB, T], dt.bfloat16, tag=f"q{h}",
                                   name=f"q{h}")
                    ep = epat_sb[:, h * HALF:(h + 1) * HALF]
                    nc.vector.tensor_mul(
                        q, ep.rearrange("p (c3 b i) -> p c3 b i", c3=4, b=NB),
                        fslice(0, h))
                    qcur.append(q)
                for s in range(1, L):
                    for h in range(2):
                        psq = psqpool.tile([P_SCAN, HALF], dt.float32, tag="psq",
                                           name="psq")
                        nc.tensor.matmul(
                            psq, lhsE_sb,
                            qcur[h].rearrange("p c3 b i -> p (c3 b i)"),
                            start=True, stop=True)
                        qn = qpool.tile([P_SCAN, 4, NB, T], dt.bfloat16,
                                        tag=f"q{h}", name=f"qn{h}")
                        nc.vector.tensor_mul(
                            qn, psq.rearrange("p (c3 b i) -> p c3 b i",
                                              c3=4, b=NB),
                            fslice(s, h))
                        qcur[h] = qn
                for h in range(2):
                    # back to f32 for the host
                    qff = fpool.tile([P_SCAN, HALF], dt.float32, tag=f"qf{h}",
                                     name=f"qf{h}")
                    nc.vector.tensor_copy(
                        qff, qcur[h].rearrange("p c3 b i -> p (c3 b i)"))
                    nc.gpsimd.dma_start(q_out[:, h * HALF:(h + 1) * HALF], qff)
            nc.gpsimd.dma_start(eT_out[:, :], eT_sb)

    nc.compile()
    return nc


def _get_nc():
    if "nc" not in _cached:
        _cached["nc"] = _build_nc()
    return _cached["nc"]


def _host_prep(hidden_states, fc_w, fc_b, start_transitions, transitions):
    """Build the 8 per-core input maps."""
    import ml_dtypes
    np_mdt = ml_dtypes.bfloat16 if MOVING_DTYPE == "bf16" else np.float32

    E = np.exp(transitions.astype(np.float64)).astype(np.float32)     # [T,T]
    # epat[(g,j),(c3,b,i)] = E[i,j], except chunk 0 (g=0, c3=0) slots = 1
    epat = np.tile(E.T[None, :, None, None, :], (NG, 1, C3, NB, 1))   # [g,j,c3,b,i]
    epat[0, :, 0, :, :] = 1.0
    epat = np.ascontiguousarray(epat.reshape(P_SCAN, NFREE), dtype=np.float32)
    # lhsE = blockdiag(E) x8: lhsT[(g,k),(g,j)] = E[k,j]  (bf16 scan matmul)
    lhsE = np.zeros((P_SCAN, P_SCAN), dtype=ml_dtypes.bfloat16)
    for g in range(NG):
        lhsE[g * T:(g + 1) * T, g * T:(g + 1) * T] = E.astype(ml_dtypes.bfloat16)
    fcwT = np.ascontiguousarray(fc_w.T.astype(np_mdt))                # [H,T]
    biasF = np.ascontiguousarray(
        np.tile(fc_b - SIGMA, NG).reshape(P_SCAN, 1), dtype=np.float32)
    bias0 = np.ascontiguousarray(
        (start_transitions + fc_b).reshape(T, 1), dtype=np.float32)

    in_maps = []
    for cid in range(NCORES):
        hc = hidden_states[cid * NB:(cid + 1) * NB]                   # [NB,S,H]
        # t-major token order: col = t*NB + b
        hc = hc.transpose(1, 0, 2).reshape(NTOK, H)
        hTc = np.ascontiguousarray(hc.T.astype(np_mdt))               # [H,4096]
        in_maps.append({
            "hT": hTc, "fcwT": fcwT, "lhsE": lhsE, "epat": epat,
            "biasF": biasF, "bias0": bias0,
        })
    return in_maps


def _host_finish(results, labels, fc_b, start_transitions,
                 end_transitions, transitions):
    """Numerator + chunk-matrix combine, all in f64."""
    labels = labels.astype(np.int64)
    start = start_transitions.astype(np.float64)
    end = end_transitions.astype(np.float64)
    trans = transitions.astype(np.float64)

    # reassemble e [B, S, T] from per-core e^T [9, 4096] (+ fc_b)
    # token order is t-major: col = t*NB + b
    e = np.empty((B, S, T), dtype=np.float64)
    for cid in range(NCORES):
        eT = results[cid]["eT_out"].astype(np.float64)    # [9, 4096]
        e[cid * NB:(cid + 1) * NB] = eT.T.reshape(S, NB, T).transpose(1, 0, 2)
    e += fc_b.astype(np.float64)

    # numerator (mask all-ones fast path)
    emit = np.take_along_axis(e, labels[..., None], axis=-1)[..., 0]
    num = start[labels[:, 0]] + emit[:, 0]
    num = num + (trans[labels[:, :-1], labels[:, 1:]] + emit[:, 1:]).sum(1)
    num = num + end[labels[:, -1]]

    # denominator: combine chunk matrices
    # chunk c = 8g + c3;  Q[(g,j), (c3,b,i)] = P_c[i, j]
    den = np.empty(B)
    for cid in range(NCORES):
        Q = results[cid]["q_out"].astype(np.float64)      # [72, 576]
        Q = Q.reshape(NG, T, C3, NB, T)                   # [g, j, c3, b, i]
        for b in range(NB):
            alpha = Q[0, :, 0, b, 0].copy()  # P_0[0,:] (rows of P_0 all equal)
            corr = 0.0
            for c in range(1, C):
                g, c3 = c // C3, c % C3
                Pc = Q[g, :, c3, b, :].T                  # P_c[i, j] rows i
                alpha = alpha @ Pc
                m = alpha.max()
                alpha /= m
                corr += np.log(m)
            den[cid * NB + b] = np.log((alpha * np.exp(end)).sum()) + corr \
                + (S - 1) * SIGMA
    return np.float32(-(num - den).sum())


def kernel(**inputs):
    hidden_states = np.asarray(inputs["hidden_states"], dtype=np.float32)
    attention_mask = np.asarray(inputs["attention_mask"])
    labels = np.asarray(inputs["labels"])
    fc_w = np.asarray(inputs["fc_w"], dtype=np.float32)
    fc_b = np.asarray(inputs["fc_b"], dtype=np.float32)
    start_transitions = np.asarray(inputs["start_transitions"], dtype=np.float32)
    end_transitions = np.asarray(inputs["end_transitions"], dtype=np.float32)
    transitions = np.asarray(inputs["transitions"], dtype=np.float32)

    if (hidden_states.shape != (B, S, H)) or not np.all(attention_mask != 0):
        return _reference_host(hidden_states, attention_mask, labels, fc_w,
                               fc_b, start_transitions, end_transitions,
                               transitions)

    from concourse.bass_utils import run_bass_kernel_spmd
    nc = _get_nc()
    in_maps = _host_prep(hidden_states, fc_w, fc_b, start_transitions,
                         transitions)
    res = run_bass_kernel_spmd(nc, in_maps, core_ids=list(range(NCORES)))
    _cached["last_res"] = res
    return _host_finish(res.results, labels, fc_b, start_transitions,
                        end_transitions, transitions)
